# revision 1
# baseline (speedup 1.0000x reference)
"""GCN (3-layer, symmetric-norm) on 8 Trainium2 NeuronCores.

Graph/data parallel via dst-vertex cut: each core owns 25000 destination
nodes and their incident edges. Nodes are relabeled into a class-sorted "pi"
order (grouped by padded-degree class, round-robin over 128 partitions) so
the per-destination segmented sum becomes regular strided DVE adds. x[src]
gathers are 128-row indirect DMAs from an AllGathered DRAM table.

vs v1: self-loops are excluded from the gather (the own contribution is a
local DVE add of the previously written-back x*dinv values); dinv is
precomputed on the host (degrees are static) and phantom ranks carry dinv=0
so phantom table rows are exactly zero -- pad gather slots point at phantom
rows, removing the indicator mask multiply and the on-device degree
reduction. Gather bodies are emitted per class so each class's reduction
overlaps the next class's gather.
"""
import numpy as np

N = 200000
NCORE = 8
SH = N // NCORE  # 25000 dst nodes per core
FD, HD = 8, 32
B = 48  # gather slots per loop body


def _plan_classes(cnt, max_deg):
    """DP: partition degrees into classes [lo..hi] (segment size hi),
    minimizing real slot padding + 128-alignment phantom slots."""
    tot = cnt.sum(axis=0)
    INF = float("inf")
    dp = [INF] * (max_deg + 2)
    choice = [0] * (max_deg + 2)
    dp[1] = 0.0
    for lo in range(1, max_deg + 1):
        if dp[lo] == INF:
            continue
        for hi in range(lo, max_deg + 1):
            pad = sum(int(tot[d]) * (hi - d) for d in range(lo, hi + 1))
            phantom = NCORE * 64.0 * hi
            c = dp[lo] + pad + phantom
            if c < dp[hi + 1]:
                dp[hi + 1] = c
                choice[hi + 1] = lo
    classes = []
    hi = max_deg + 1
    while hi > 1:
        lo = choice[hi]
        classes.append((lo, hi - 1))
        hi = lo
    classes.reverse()
    return classes


def _preprocess(inputs):
    src = np.asarray(inputs["edge_index"][0]).astype(np.int64)
    dst = np.asarray(inputs["edge_index"][1]).astype(np.int64)
    deg_ns = np.bincount(dst, minlength=N)       # no self loop
    deg_full = deg_ns + 1                        # reference adds self loops
    dinv_full = 1.0 / np.sqrt(deg_full.astype(np.float64))
    deg_l = np.maximum(deg_ns, 1)                # layout degree (slots)
    max_deg = int(deg_l.max())

    cnt = np.zeros((NCORE, max_deg + 1), np.int64)
    for c in range(NCORE):
        cnt[c] = np.bincount(deg_l[c * SH:(c + 1) * SH], minlength=max_deg + 1)

    classes = _plan_classes(cnt, max_deg)
    ncls = len(classes)
    cls_of_deg = np.zeros(max_deg + 1, np.int64)
    seg = np.zeros(ncls, np.int64)
    for i, (lo, hi) in enumerate(classes):
        cls_of_deg[lo:hi + 1] = i
        seg[i] = hi
    cls_cnt = np.zeros((NCORE, ncls), np.int64)
    for c in range(NCORE):
        for i, (lo, hi) in enumerate(classes):
            cls_cnt[c, i] = cnt[c, lo:hi + 1].sum()
    M_cls = cls_cnt.max(axis=0)
    G_cls = (M_cls + 127) // 128
    # ensure at least one phantom rank exists on every core (pad target)
    if np.all(G_cls * 128 == M_cls):
        G_cls[-1] += 1
    Gtot = int(G_cls.sum())
    Jn = Gtot * 128
    node_off = np.concatenate([[0], np.cumsum(G_cls)]) * 128
    col_off = np.concatenate([[0], np.cumsum(G_cls * seg)])
    J = int(col_off[-1])
    Jpad = (J + B - 1) // B * B

    # rank (p=rank%128, g=rank//128) -> table row is PARTITION-MAJOR
    # (row = p*Gtot + g) so the writeback DMA is one contiguous run per
    # partition; featU keeps the g-major order for the embed matmul.
    pirow = np.zeros(N, np.int64)
    rank_in_core = np.zeros(N, np.int64)
    phantom_rank = np.zeros(NCORE, np.int64)
    for c in range(NCORE):
        nodes = np.arange(c * SH, (c + 1) * SH)
        cl = cls_of_deg[deg_l[nodes]]
        order = np.lexsort((nodes, cl))
        snodes = nodes[order]
        scl = cl[order]
        ranks = np.zeros(SH, np.int64)
        used = np.zeros(Jn, bool)
        for i in range(ncls):
            m = scl == i
            ranks[m] = node_off[i] + np.arange(int(m.sum()))
        used[ranks] = True
        ph = np.flatnonzero(~used)
        assert len(ph) > 0
        phantom_rank[c] = ph[0]
        rank_in_core[snodes] = ranks
        pirow[snodes] = c * Jn + (ranks % 128) * Gtot + ranks // 128

    # gather slots: pad -> a phantom row of the local core (zero by dinv=0)
    idx_g = np.zeros((NCORE, 128, Jpad), np.int32)
    for c in range(NCORE):
        phr = phantom_rank[c]
        pr = c * Jn + (phr % 128) * Gtot + phr // 128
        idx_g[c, :, :] = pr
    order_e = np.lexsort((src, dst))
    src_s, dst_s = src[order_e], dst[order_e]
    estart = np.searchsorted(dst_s, np.arange(N))
    e_core = dst_s // SH
    e_rank = rank_in_core[dst_s]
    e_cls = cls_of_deg[deg_l[dst_s]]
    e_p = e_rank % 128
    e_g = (e_rank - node_off[e_cls]) // 128
    e_col = col_off[e_cls] + e_g * seg[e_cls] + \
        (np.arange(len(src_s)) - estart[dst_s])
    idx_g[e_core, e_p, e_col] = pirow[src_s].astype(np.int32)

    # dinv in rank layout; phantom ranks -> 0
    dinvT = np.zeros((NCORE, 128, Gtot), np.float32)
    p_all = rank_in_core % 128
    g_all = rank_in_core // 128
    cores_all = np.arange(N) // SH
    dinvT[cores_all, p_all, g_all] = dinv_full.astype(np.float32)

    # unified typed features, pi-ordered; phantom rows zero
    featU = np.zeros((NCORE, Jn, 50), np.float32)
    rows_c = rank_in_core            # g-major row within core (embed order)
    cores_n = np.arange(N) // SH
    t_ev, t_cs, t_tr = 100000, 160000, 199999
    nodes = np.arange(N)
    for lo, hi, key, fo, oh in ((0, t_ev, "ev_features", 0, 46),
                                (t_ev, t_cs, "cs_features", 16, 47),
                                (t_cs, t_tr, "tr_features", 28, 48),
                                (t_tr, N, "env_features", 36, 49)):
        feat = np.asarray(inputs[key], np.float32)
        nn = nodes[lo:hi]
        featU[cores_n[nn], rows_c[nn], fo:fo + feat.shape[1]] = feat
        featU[cores_n[nn], rows_c[nn], oh] = 1.0

    Wcomb = np.zeros((50, FD), np.float32)
    Wcomb[0:16] = np.asarray(inputs["W_ev"], np.float32)
    Wcomb[16:28] = np.asarray(inputs["W_cs"], np.float32)
    Wcomb[28:36] = np.asarray(inputs["W_tr"], np.float32)
    Wcomb[36:46] = np.asarray(inputs["W_env"], np.float32)
    Wcomb[46] = np.asarray(inputs["b_ev"], np.float32)
    Wcomb[47] = np.asarray(inputs["b_cs"], np.float32)
    Wcomb[48] = np.asarray(inputs["b_tr"], np.float32)
    Wcomb[49] = np.asarray(inputs["b_env"], np.float32)

    plan = dict(classes=classes, seg=seg, G_cls=G_cls, Gtot=Gtot, Jn=Jn,
                J=J, Jpad=Jpad, col_off=col_off, node_off=node_off)
    return plan, idx_g, dinvT, featU, Wcomb, pirow


def _build_program(plan):
    import concourse.bacc as bacc
    import concourse.tile as tile
    import concourse.mybir as mybir
    import concourse.bass as bass

    Gtot, Jn, Jpad = plan["Gtot"], plan["Jn"], plan["Jpad"]
    classes, seg = plan["classes"], plan["seg"]
    G_cls, col_off, node_off = plan["G_cls"], plan["col_off"], plan["node_off"]
    dt = mybir.dt
    AF = mybir.ActivationFunctionType

    nc = bacc.Bacc("TRN2", target_bir_lowering=False, debug=False,
                   num_devices=NCORE, dynamic_dma_scratch_size=24576)
    featUT_in = nc.dram_tensor("featUT", [50, Jn], dt.float32,
                               kind="ExternalInput").ap()
    idx_in = nc.dram_tensor("idxg", [128, Jpad], dt.int32,
                            kind="ExternalInput").ap()
    dinv_in = nc.dram_tensor("dinvt", [128, Gtot], dt.float32,
                             kind="ExternalInput").ap()
    wcomb_in = nc.dram_tensor("wcomb", [50, FD], dt.float32,
                              kind="ExternalInput").ap()
    NW = 8 * 32 + 32 + 32 * 8 + 8 + 8 * 8 + 8
    wrows_in = nc.dram_tensor("wrows", [128, NW], dt.float32,
                              kind="ExternalInput").ap()
    out_d = nc.dram_tensor("outv", [Jn], dt.float32, kind="ExternalOutput").ap()

    with tile.TileContext(nc) as tc:
        with tc.tile_pool(name="sbuf", bufs=1) as pool, \
             tc.tile_pool(name="fslab", bufs=2) as fpool, \
             tc.tile_pool(name="stage", bufs=6) as stage, \
             tc.tile_pool(name="psum", bufs=8, space="PSUM") as psum, \
             tc.tile_pool(name="dram", bufs=1, space="DRAM") as dram:

            table0 = dram.tile([NCORE * Jn, FD], dt.float32,
                               addr_space="Shared", tag="tbl0")
            table1 = dram.tile([NCORE * Jn, FD], dt.float32,
                               addr_space="Shared", tag="tbl1")
            table2 = dram.tile([NCORE * Jn, FD], dt.float32,
                               addr_space="Shared", tag="tbl2")
            shard0 = dram.tile([Jn, FD], dt.float32, tag="shd0")
            shard1 = dram.tile([Jn, FD], dt.float32, tag="shd1")
            shard2 = dram.tile([Jn, FD], dt.float32, tag="shd2")
            tables = [table0, table1, table2]
            shards = [shard0, shard1, shard2]

            idx = pool.tile([128, Jpad], dt.int32)
            nc.sync.dma_start(out=idx[:], in_=idx_in[:])
            slab = pool.tile([128, Jpad * FD], dt.float32)
            xcur = pool.tile([128, Gtot * HD], dt.float32)
            agg = pool.tile([128, Gtot * FD], dt.float32)
            ycur = pool.tile([128, Gtot * FD], dt.float32)
            dinv = pool.tile([128, Gtot], dt.float32)
            fin = pool.tile([128, Gtot], dt.float32)
            tmat = pool.tile([128, 32 * HD], dt.float32)
            wc = pool.tile([50, FD], dt.float32)
            wr = pool.tile([128, NW], dt.float32)
            nc.sync.dma_start(out=dinv[:], in_=dinv_in[:])
            nc.sync.dma_start(out=wc[:], in_=wcomb_in[:])
            nc.sync.dma_start(out=wr[:], in_=wrows_in[:])
            cW0 = wr[:, 0:256]
            cb0 = wr[:, 256:288]
            cW1 = wr[:, 288:544]
            cb1 = wr[:, 544:552]
            cb2 = wr[:, 616:617]

            # ---------------- embed: x0 = relu(featU @ Wcomb) ----------------
            CH_PER_SLAB = 4
            for sb in range(0, Gtot, CH_PER_SLAB):
                nch = min(CH_PER_SLAB, Gtot - sb)
                fsl = fpool.tile([50, CH_PER_SLAB * 128], dt.float32, tag="fsl")
                nc.sync.dma_start(
                    out=fsl[:, :nch * 128],
                    in_=featUT_in[:, sb * 128: (sb + nch) * 128])
                for c2 in range(nch):
                    ch = sb + c2
                    pt = psum.tile([128, FD], dt.float32, tag="emb")
                    nc.tensor.matmul(out=pt[:],
                                     lhsT=fsl[:, c2 * 128:(c2 + 1) * 128],
                                     rhs=wc[:], start=True, stop=True)
                    nc.scalar.activation(out=agg[:, ch * FD:(ch + 1) * FD],
                                         in_=pt[:], func=AF.Relu)

            # class-structured segmented reduction for ONE class region
            def reduce_one_class(buf, W, i):
                s = int(seg[i])
                ng = int(G_cls[i])
                co = int(col_off[i])
                reg = buf[:, co * W:(co + ng * s) * W].rearrange(
                    "p (g c) -> p g c", g=ng)
                w = s
                while w > 1:
                    if w % 2 == 1:
                        nc.vector.tensor_add(
                            out=reg[:, :, 0:W], in0=reg[:, :, 0:W],
                            in1=reg[:, :, (w - 1) * W:w * W])
                        w -= 1
                    h = w // 2
                    if h > 0 and w > 1:
                        nc.vector.tensor_add(
                            out=reg[:, :, 0:h * W], in0=reg[:, :, 0:h * W],
                            in1=reg[:, :, h * W:2 * h * W])
                    w = h

            def compact_one_class(buf, W, out, i):
                s = int(seg[i])
                ng = int(G_cls[i])
                co = int(col_off[i])
                no = int(node_off[i]) // 128
                src3 = buf[:, co * W:(co + ng * s) * W].rearrange(
                    "p (g c) -> p g c", g=ng)[:, :, 0:W]
                dst3 = out[:, no * W:(no + ng) * W].rearrange(
                    "p (g c) -> p g c", g=ng)
                nc.vector.tensor_copy(out=dst3, in_=src3)

            def scale_by_dinv(region, W):
                d3 = dinv[:].rearrange("p (g o) -> p g o", o=1).to_broadcast(
                    [128, Gtot, W])
                s3 = region.rearrange("p (g w) -> p g w", w=W)
                nc.vector.tensor_mul(out=s3, in0=s3, in1=d3)

            def writeback_allgather(region, li):
                shard_ap = shards[li][:].rearrange("(p g) f -> p g f", g=Gtot)
                nc.sync.dma_start(
                    out=shard_ap, in_=region.rearrange("p (g f) -> p g f", f=FD))
                nc.gpsimd.collective_compute(
                    "AllGather", mybir.AluOpType.bypass,
                    replica_groups=[list(range(NCORE))],
                    ins=[shards[li].opt()], outs=[tables[li].opt()])

            # gather + reduce interleaved per class: class i's reduction and
            # tail (matmuls/writeback chunk) run on DVE while class i+1's
            # slots are still being gathered on GpSimd/DMA
            def gather_reduce(li, out, tail=None):
                nb_done = 0
                for i in range(len(classes)):
                    c1 = int(col_off[i] + G_cls[i] * seg[i])
                    nb0 = nb_done
                    nb1 = (c1 + B - 1) // B
                    if i == len(classes) - 1:
                        nb1 = Jpad // B
                    nb_done = nb1

                    def gbody(it):
                        ib = stage.tile([128, B], dt.int32, tag="ib")
                        ob = stage.tile([128, B * FD], dt.float32, tag="ob")
                        nc.vector.tensor_copy(out=ib[:],
                                              in_=idx[:, bass.ds(it * B, B)])
                        for j in range(B):
                            nc.gpsimd.indirect_dma_start(
                                out=ob[:, j * FD:(j + 1) * FD], out_offset=None,
                                in_=tables[li][:],
                                in_offset=bass.IndirectOffsetOnAxis(
                                    ap=ib[:, j:j + 1], axis=0))
                        nc.vector.tensor_copy(
                            out=slab[:, bass.ds(it * (B * FD), B * FD)],
                            in_=ob[:])
                    if nb1 > nb0:
                        tc.For_i_unrolled(nb0, nb1, 1, gbody, max_unroll=4)
                    reduce_one_class(slab, FD, i)
                    compact_one_class(slab, FD, out, i)
                    if tail is not None:
                        tail(int(node_off[i]) // 128, int(G_cls[i]))
                if tail is None:
                    # self-loop: own x*dinv contribution
                    nc.vector.tensor_add(out=out[:, :Gtot * FD],
                                         in0=out[:, :Gtot * FD], in1=ycur[:])

            # ---- chunked (per rank-group range) elementwise helpers ----
            TM = 32  # groups per sub-chunk (scratch-bounded)

            def cview(t, W, c0, nch):
                return t[:, c0 * W:(c0 + nch) * W].rearrange(
                    "p (g w) -> p g w", w=W)

            def cdinv(W, c0, nch):
                return dinv[:, c0:c0 + nch].rearrange(
                    "p (g o) -> p g o", o=1).to_broadcast([128, nch, W])

            def cadd_ycur(region, c0, nch):
                r3 = cview(region, FD, c0, nch)
                nc.vector.tensor_add(out=r3, in0=r3,
                                     in1=cview(ycur, FD, c0, nch))

            def cscale(region, W, c0, nch):
                r3 = cview(region, W, c0, nch)
                nc.vector.tensor_mul(out=r3, in0=r3, in1=cdinv(W, c0, nch))

            def cmatmul(src, Fin, Fout, wap, out, c0, nch):
                o3 = cview(out, Fout, c0, nch)
                t3 = tmat[:, :nch * Fout].rearrange("p (g w) -> p g w", w=Fout)
                s3 = cview(src, Fin, c0, nch)
                for fi in range(Fin):
                    sb = s3[:, :, fi:fi + 1].to_broadcast([128, nch, Fout])
                    wrow = wap[:, fi * Fout:(fi + 1) * Fout].rearrange(
                        "p (o w) -> p o w", o=1).to_broadcast([128, nch, Fout])
                    if fi == 0:
                        nc.vector.tensor_mul(out=o3, in0=sb, in1=wrow)
                    else:
                        nc.vector.tensor_mul(out=t3, in0=sb, in1=wrow)
                        nc.vector.tensor_add(out=o3, in0=o3, in1=t3)

            def cbias_relu(region, W, bap, c0, nch, relu=True):
                r3 = cview(region, W, c0, nch)
                bb = bap.rearrange("p (o w) -> p o w", o=1).to_broadcast(
                    [128, nch, W])
                nc.vector.tensor_add(out=r3, in0=r3, in1=bb)
                if relu:
                    nc.vector.tensor_relu(out=r3, in_=r3)

            def cwriteback(region, li, no, ng):
                shard_ap = shards[li][:].rearrange(
                    "(p g) f -> p g f", g=Gtot)[:, no:no + ng, :]
                nc.sync.dma_start(out=shard_ap,
                                  in_=cview(region, FD, no, ng))

            def tail_L0(no, ng):
                for c0 in range(no, no + ng, TM):
                    nch = min(TM, no + ng - c0)
                    cadd_ycur(agg, c0, nch)
                    cscale(agg, FD, c0, nch)
                    cmatmul(agg, FD, HD, cW0, xcur, c0, nch)
                    cbias_relu(xcur, HD, cb0, c0, nch)
                    cmatmul(xcur, HD, FD, cW1, agg, c0, nch)
                    cscale(agg, FD, c0, nch)
                    nc.vector.tensor_copy(out=cview(ycur, FD, c0, nch),
                                          in_=cview(agg, FD, c0, nch))
                cwriteback(agg, 1, no, ng)

            def tail_L1(no, ng):
                for c0 in range(no, no + ng, TM):
                    nch = min(TM, no + ng - c0)
                    cadd_ycur(agg, c0, nch)
                    cscale(agg, FD, c0, nch)
                    cbias_relu(agg, FD, cb1, c0, nch)
                    cmatmul(agg, FD, FD, wr[:, 552:616], xcur, c0, nch)
                    cscale(xcur, FD, c0, nch)
                    nc.vector.tensor_copy(out=cview(ycur, FD, c0, nch),
                                          in_=cview(xcur, FD, c0, nch))
                cwriteback(xcur, 2, no, ng)

            def collective(li):
                nc.gpsimd.collective_compute(
                    "AllGather", mybir.AluOpType.bypass,
                    replica_groups=[list(range(NCORE))],
                    ins=[shards[li].opt()], outs=[tables[li].opt()])

            # y0 = dinv * x0 -> ycur, table
            x0r = agg[:, :Gtot * FD]
            scale_by_dinv(x0r, FD)
            nc.vector.tensor_copy(out=ycur[:], in_=x0r)
            writeback_allgather(x0r, 0)

            # ---------------- Layer 0 ----------------
            # per-class tail computes x1 = relu(dinv*agg@W0+b0), t1 = x1@W1,
            # y1 = dinv*t1, and writes its shard chunk; collective at the end
            gather_reduce(0, agg, tail=tail_L0)
            collective(1)

            # ---------------- Layer 1 ----------------
            gather_reduce(1, agg, tail=tail_L1)
            collective(2)

            # ---------------- Layer 2 ----------------
            gather_reduce(2, xcur)
            aggr3 = xcur[:, :Gtot * FD].rearrange("p (g w) -> p g w", w=FD)
            nc.vector.tensor_copy(out=fin[:].rearrange("p (g o) -> p g o", o=1),
                                  in_=aggr3[:, :, 0:1])
            nc.vector.tensor_mul(out=fin[:], in0=fin[:], in1=dinv[:])
            nc.scalar.activation(out=fin[:], in_=fin[:], func=AF.Tanh,
                                 bias=cb2)
            nc.sync.dma_start(out=out_d.rearrange("(p g) -> p g", g=Gtot),
                              in_=fin[:])
    nc.compile()
    return nc


def kernel(**inputs):
    from concourse.bass_utils import run_bass_kernel_spmd

    plan, idx_g, dinvT, featU, Wcomb, pirow = _preprocess(inputs)
    W0 = np.asarray(inputs["W0"], np.float32)
    b0 = np.asarray(inputs["b0"], np.float32)
    W1 = np.asarray(inputs["W1"], np.float32)
    b1 = np.asarray(inputs["b1"], np.float32)
    W2 = np.asarray(inputs["W2"], np.float32)
    b2 = np.asarray(inputs["b2"], np.float32)

    wrows = np.concatenate([W0.reshape(-1), b0, W1.reshape(-1), b1,
                            np.pad(W2[:, 0:1], ((0, 0), (0, 7))).reshape(-1),
                            np.pad(b2, (0, 7))])
    wrows_t = np.tile(wrows[None, :], (128, 1)).astype(np.float32)

    nc = _build_program(plan)

    in_maps = []
    for c in range(NCORE):
        in_maps.append({
            "featUT": np.ascontiguousarray(featU[c].T),
            "idxg": idx_g[c],
            "dinvt": dinvT[c],
            "wcomb": Wcomb,
            "wrows": wrows_t,
        })
    trace = False
    try:
        from antenv.axon_hooks import get_axon_ntff_profile_hook
        trace = get_axon_ntff_profile_hook() is not None
    except Exception:
        trace = False
    res = run_bass_kernel_spmd(nc, in_maps, list(range(NCORE)), trace=trace)
    global LAST_EXEC_NS, LAST_RES
    LAST_EXEC_NS = res.exec_time_ns
    LAST_RES = res
    allv = np.concatenate([res.results[c]["outv"] for c in range(NCORE)])
    return allv[pirow].astype(np.float32)



# revision 4
# speedup vs baseline: 1.3712x; 1.3712x over previous
"""GCN (3-layer, symmetric-norm) on 8 Trainium2 NeuronCores.

Graph/data parallel via dst-vertex cut: each core owns 25000 destination
nodes and their incident edges. Nodes are relabeled into a class-sorted "pi"
order (grouped by padded-degree class, round-robin over 128 partitions) so
the per-destination segmented sum becomes regular strided DVE adds.

v3: the per-slot x[src] delivery uses hardware dma_gather (InstDMAGatherAnt,
mlp ucode library) instead of per-column indirect DMAs. Each slot gathers
the 256B block of 8 consecutive table rows containing its src (block id
fits int16), on 4 SWDGE queues in parallel (~3ns/descriptor vs ~11ns for
the single-queue indirect path). A DVE mask-select (one-hot over the 8
subrows, masks from an is_equal against a host-built sub-id plane) extracts
the 8-float src row; pads get sub-id 255 so they select to exactly zero.
Selection output is reduced chunk-by-chunk (64-column chunks) straight into
the per-dst accumulator, so only a 2x16KB chunk window is SBUF-resident.
"""
import numpy as np

N = 200000
NCORE = 8
SH = N // NCORE  # 25000 dst nodes per core
FD, HD = 8, 32
CH = 64          # slot columns per gather/select chunk
NIB = 1024       # idxs per dma_gather instruction (8 columns)
NQ = 4           # SWDGE queues


def _plan_classes(cnt, max_deg):
    """DP: partition degrees into classes [lo..hi] (segment size hi),
    minimizing real slot padding + per-class fixed overhead."""
    tot = cnt.sum(axis=0)
    INF = float("inf")
    dp = [INF] * (max_deg + 2)
    choice = [0] * (max_deg + 2)
    dp[1] = 0.0
    for lo in range(1, max_deg + 1):
        if dp[lo] == INF:
            continue
        for hi in range(lo, max_deg + 1):
            pad = sum(int(tot[d]) * (hi - d) for d in range(lo, hi + 1))
            fixed = NCORE * 64.0 * hi + 8192.0
            c = dp[lo] + pad + fixed
            if c < dp[hi + 1]:
                dp[hi + 1] = c
                choice[hi + 1] = lo
    classes = []
    hi = max_deg + 1
    while hi > 1:
        lo = choice[hi]
        classes.append((lo, hi - 1))
        hi = lo
    # descending segment size: the smallest class last so the Gtot%8
    # padding (appended to the last class) is cheap
    classes.sort(key=lambda c: -c[1])
    return classes


def _preprocess(inputs):
    src = np.asarray(inputs["edge_index"][0]).astype(np.int64)
    dst = np.asarray(inputs["edge_index"][1]).astype(np.int64)
    deg_ns = np.bincount(dst, minlength=N)       # no self loop
    deg_full = deg_ns + 1                        # reference adds self loops
    dinv_full = 1.0 / np.sqrt(deg_full.astype(np.float64))
    deg_l = np.maximum(deg_ns, 1)                # layout degree (slots)
    max_deg = int(deg_l.max())

    cnt = np.zeros((NCORE, max_deg + 1), np.int64)
    for c in range(NCORE):
        cnt[c] = np.bincount(deg_l[c * SH:(c + 1) * SH], minlength=max_deg + 1)

    classes = _plan_classes(cnt, max_deg)
    ncls = len(classes)
    cls_of_deg = np.zeros(max_deg + 1, np.int64)
    seg = np.zeros(ncls, np.int64)
    for i, (lo, hi) in enumerate(classes):
        cls_of_deg[lo:hi + 1] = i
        seg[i] = hi
    cls_cnt = np.zeros((NCORE, ncls), np.int64)
    for c in range(NCORE):
        for i, (lo, hi) in enumerate(classes):
            cls_cnt[c, i] = cnt[c, lo:hi + 1].sum()
    M_cls = cls_cnt.max(axis=0)
    G_cls = (M_cls + 127) // 128
    # block alignment: Gtot must be a multiple of 8 (8-row 256B blocks
    # must not straddle partitions); pad the last (smallest-seg) class
    rem = int(G_cls.sum()) % 8
    if rem:
        G_cls[-1] += 8 - rem
    Gtot = int(G_cls.sum())
    Jn = Gtot * 128
    node_off = np.concatenate([[0], np.cumsum(G_cls)]) * 128
    # class column regions, each padded to a whole number of CH-chunks
    ccols = ((G_cls * seg + CH - 1) // CH) * CH
    col_off = np.concatenate([[0], np.cumsum(ccols)])
    Jpad = int(col_off[-1])

    # rank (p=rank%128, g=rank//128) -> table row is PARTITION-MAJOR
    # (row = p*Gtot + g) so the writeback DMA is one contiguous run per
    # partition; featU keeps the g-major order for the embed matmul.
    pirow = np.zeros(N, np.int64)
    rank_in_core = np.zeros(N, np.int64)
    for c in range(NCORE):
        nodes = np.arange(c * SH, (c + 1) * SH)
        cl = cls_of_deg[deg_l[nodes]]
        order = np.lexsort((nodes, cl))
        snodes = nodes[order]
        scl = cl[order]
        ranks = np.zeros(SH, np.int64)
        for i in range(ncls):
            m = scl == i
            ranks[m] = node_off[i] + np.arange(int(m.sum()))
        rank_in_core[snodes] = ranks
        pirow[snodes] = c * Jn + (ranks % 128) * Gtot + ranks // 128

    # per-slot gather block (int16, = pirow//8) and sub-id plane
    # (float32: 0..7 real, 255 pad -> masks select exactly zero)
    blk = np.zeros((NCORE, 128, Jpad), np.int16)
    sub = np.full((NCORE, 128, Jpad), 255.0, np.float32)
    order_e = np.lexsort((src, dst))
    src_s, dst_s = src[order_e], dst[order_e]
    estart = np.searchsorted(dst_s, np.arange(N))
    e_core = dst_s // SH
    e_rank = rank_in_core[dst_s]
    e_cls = cls_of_deg[deg_l[dst_s]]
    e_p = e_rank % 128
    e_g = (e_rank - node_off[e_cls]) // 128
    e_col = col_off[e_cls] + e_g * seg[e_cls] + \
        (np.arange(len(src_s)) - estart[dst_s])
    pr = pirow[src_s]
    blk[e_core, e_p, e_col] = (pr // 8).astype(np.int16)
    sub[e_core, e_p, e_col] = (pr % 8).astype(np.float32)

    # idx list in the dma_gather wrap layout: instruction m covers slot
    # columns [8m, 8m+8) on queue m%4 (queue-local index jq=m//4); list
    # element i (= slot (i%128, 8m + i//128)) is stored at
    # [32q+16 + i%16, 64*jq + i//16].
    NINST = Jpad // 8
    NINSTQ = NINST // NQ
    idx16 = np.zeros((NCORE, 128, NINSTQ * 64), np.int16)
    for m in range(NINST):
        q, jq = m % NQ, m // NQ
        cols = blk[:, :, 8 * m:8 * m + 8]            # [NCORE, 128, 8]
        lst = cols.transpose(0, 2, 1).reshape(NCORE, NIB)  # i = c*128+p
        idx16[:, 32 * q + 16:32 * q + 32, 64 * jq:64 * jq + 64] = \
            lst.reshape(NCORE, 64, 16).transpose(0, 2, 1)
    # dinv in rank layout; unused ranks -> 0
    dinvT = np.zeros((NCORE, 128, Gtot), np.float32)
    p_all = rank_in_core % 128
    g_all = rank_in_core // 128
    cores_all = np.arange(N) // SH
    dinvT[cores_all, p_all, g_all] = dinv_full.astype(np.float32)

    # unified typed features, pi-ordered; unused rows zero
    featU = np.zeros((NCORE, Jn, 50), np.float32)
    rows_c = rank_in_core            # g-major row within core (embed order)
    cores_n = np.arange(N) // SH
    t_ev, t_cs, t_tr = 100000, 160000, 199999
    nodes = np.arange(N)
    for lo, hi, key, fo, oh in ((0, t_ev, "ev_features", 0, 46),
                                (t_ev, t_cs, "cs_features", 16, 47),
                                (t_cs, t_tr, "tr_features", 28, 48),
                                (t_tr, N, "env_features", 36, 49)):
        feat = np.asarray(inputs[key], np.float32)
        nn = nodes[lo:hi]
        featU[cores_n[nn], rows_c[nn], fo:fo + feat.shape[1]] = feat
        featU[cores_n[nn], rows_c[nn], oh] = 1.0

    Wcomb = np.zeros((50, FD), np.float32)
    Wcomb[0:16] = np.asarray(inputs["W_ev"], np.float32)
    Wcomb[16:28] = np.asarray(inputs["W_cs"], np.float32)
    Wcomb[28:36] = np.asarray(inputs["W_tr"], np.float32)
    Wcomb[36:46] = np.asarray(inputs["W_env"], np.float32)
    Wcomb[46] = np.asarray(inputs["b_ev"], np.float32)
    Wcomb[47] = np.asarray(inputs["b_cs"], np.float32)
    Wcomb[48] = np.asarray(inputs["b_tr"], np.float32)
    Wcomb[49] = np.asarray(inputs["b_env"], np.float32)

    assert NCORE * Jn // 8 <= 32767, f"block ids overflow int16: Gtot={Gtot}"
    plan = dict(classes=classes, seg=seg, G_cls=G_cls, Gtot=Gtot, Jn=Jn,
                Jpad=Jpad, col_off=col_off, node_off=node_off, ccols=ccols)
    return plan, idx16, sub, dinvT, featU, Wcomb, pirow


def _build_program(plan):
    import concourse.bacc as bacc
    import concourse.tile as tile
    import concourse.mybir as mybir
    import concourse.bass as bass

    Gtot, Jn, Jpad = plan["Gtot"], plan["Jn"], plan["Jpad"]
    classes, seg, ccols = plan["classes"], plan["seg"], plan["ccols"]
    G_cls, col_off, node_off = plan["G_cls"], plan["col_off"], plan["node_off"]
    NBLK = NCORE * Jn // 8
    dt = mybir.dt
    AF = mybir.ActivationFunctionType
    AO = mybir.AluOpType

    nc = bacc.Bacc("TRN2", target_bir_lowering=False, debug=False,
                   num_devices=NCORE, dynamic_dma_scratch_size=24576,
                   num_swdge_queues=NQ)
    featUT_in = nc.dram_tensor("featUT", [50, Jn], dt.float32,
                               kind="ExternalInput").ap()
    NINSTQ = Jpad // 8 // NQ
    idx_in = nc.dram_tensor("idx16", [128, NINSTQ * 64], dt.int16,
                            kind="ExternalInput").ap()
    sub_in = nc.dram_tensor("subid", [128, Jpad], dt.float32,
                            kind="ExternalInput").ap()
    dinv_in = nc.dram_tensor("dinvt", [128, Gtot], dt.float32,
                             kind="ExternalInput").ap()
    wcomb_in = nc.dram_tensor("wcomb", [50, FD], dt.float32,
                              kind="ExternalInput").ap()
    NW = 8 * 32 + 32 + 32 * 8 + 8 + 8 * 8 + 8
    wrows_in = nc.dram_tensor("wrows", [128, NW], dt.float32,
                              kind="ExternalInput").ap()
    out_d = nc.dram_tensor("outv", [Jn], dt.float32, kind="ExternalOutput").ap()

    with tile.TileContext(nc) as tc:
        with tc.tile_pool(name="sbuf", bufs=1) as pool, \
             tc.tile_pool(name="fslab", bufs=2) as fpool, \
             tc.tile_pool(name="ckpool", bufs=3) as ckpool, \
             tc.tile_pool(name="stage", bufs=4) as stage, \
             tc.tile_pool(name="psum", bufs=8, space="PSUM") as psum, \
             tc.tile_pool(name="dram", bufs=1, space="DRAM") as dram:

            # tables viewed as 8-row 256B blocks for dma_gather
            table0 = dram.tile([NBLK, 64], dt.float32,
                               addr_space="Shared", tag="tbl0")
            table1 = dram.tile([NBLK, 64], dt.float32,
                               addr_space="Shared", tag="tbl1")
            table2 = dram.tile([NBLK, 64], dt.float32,
                               addr_space="Shared", tag="tbl2")
            shard0 = dram.tile([Jn, FD], dt.float32, tag="shd0")
            shard1 = dram.tile([Jn, FD], dt.float32, tag="shd1")
            shard2 = dram.tile([Jn, FD], dt.float32, tag="shd2")
            tables = [table0, table1, table2]
            shards = [shard0, shard1, shard2]

            idx = pool.tile([128, NINSTQ * 64], dt.int16)
            nc.sync.dma_start(out=idx[:], in_=idx_in[:])
            subid = pool.tile([128, Jpad], dt.float32)
            nc.sync.dma_start(out=subid[:], in_=sub_in[:])
            xcur = pool.tile([128, Gtot * HD], dt.float32)
            agg = pool.tile([128, Gtot * FD], dt.float32)
            ycur = pool.tile([128, Gtot * FD], dt.float32)
            dinv = pool.tile([128, Gtot], dt.float32)
            fin = pool.tile([128, Gtot], dt.float32)
            tmat = pool.tile([128, 32 * HD], dt.float32)
            wc = pool.tile([50, FD], dt.float32)
            wr = pool.tile([128, NW], dt.float32)
            nc.sync.dma_start(out=dinv[:], in_=dinv_in[:])
            nc.sync.dma_start(out=wc[:], in_=wcomb_in[:])
            nc.sync.dma_start(out=wr[:], in_=wrows_in[:])
            cW0 = wr[:, 0:256]
            cb0 = wr[:, 256:288]
            cW1 = wr[:, 288:544]
            cb1 = wr[:, 544:552]
            cb2 = wr[:, 616:617]

            # ---------------- embed: x0 = relu(featU @ Wcomb) ----------------
            CH_PER_SLAB = 4
            for sb in range(0, Gtot, CH_PER_SLAB):
                nch = min(CH_PER_SLAB, Gtot - sb)
                fsl = fpool.tile([50, CH_PER_SLAB * 128], dt.float32, tag="fsl")
                nc.sync.dma_start(
                    out=fsl[:, :nch * 128],
                    in_=featUT_in[:, sb * 128: (sb + nch) * 128])
                for c2 in range(nch):
                    chx = sb + c2
                    pt = psum.tile([128, FD], dt.float32, tag="emb")
                    nc.tensor.matmul(out=pt[:],
                                     lhsT=fsl[:, c2 * 128:(c2 + 1) * 128],
                                     rhs=wc[:], start=True, stop=True)
                    nc.scalar.activation(out=agg[:, chx * FD:(chx + 1) * FD],
                                         in_=pt[:], func=AF.Relu)

            def scale_by_dinv(region, W):
                d3 = dinv[:].rearrange("p (g o) -> p g o", o=1).to_broadcast(
                    [128, Gtot, W])
                s3 = region.rearrange("p (g w) -> p g w", w=W)
                nc.vector.tensor_mul(out=s3, in0=s3, in1=d3)

            def collective(li):
                nc.gpsimd.collective_compute(
                    "AllGather", mybir.AluOpType.bypass,
                    replica_groups=[list(range(NCORE))],
                    ins=[shards[li].opt()], outs=[tables[li].opt()])

            def writeback_allgather(region, li):
                shard_ap = shards[li][:].rearrange("(p g) f -> p g f", g=Gtot)
                nc.sync.dma_start(
                    out=shard_ap, in_=region.rearrange("p (g f) -> p g f", f=FD))
                collective(li)

            # tree-sum columns [0,w) of a [128, n, w*FD] strided view into
            # column 0 (in place)
            def tree_sum(view3, n, w, colstride):
                # view3: AP [128, n, colstride*FD] -> operate on col slices
                while w > 1:
                    if w % 2 == 1:
                        nc.vector.tensor_add(
                            out=view3[:, :, 0:FD], in0=view3[:, :, 0:FD],
                            in1=view3[:, :, (w - 1) * FD:w * FD])
                        w -= 1
                    h = w // 2
                    if h > 0 and w > 1:
                        nc.vector.tensor_add(
                            out=view3[:, :, 0:h * FD], in0=view3[:, :, 0:h * FD],
                            in1=view3[:, :, h * FD:2 * h * FD])
                    w = h

            # ---- gather + select + reduce one layer ----
            def gather_layer(li, out, tail=None):
                nc.vector.memset(out[:, :Gtot * FD], 0.0)
                out3 = out[:, :Gtot * FD].rearrange("p (g f) -> p g f", f=FD)
                for i in range(len(classes)):
                    s = int(seg[i])
                    ng = int(G_cls[i])
                    co = int(col_off[i])
                    go = int(node_off[i]) // 128
                    creal = co + ng * s
                    cend = co + int(ccols[i])
                    for c0 in range(co, cend, CH):
                        m0 = c0 // 8
                        ck = ckpool.tile([128, CH * 64], dt.float32, tag="ck")
                        for mm in range(8):
                            m = m0 + mm
                            q, jq = m % NQ, m // NQ
                            g3 = ck[:, mm * 8 * 64:(mm + 1) * 8 * 64].rearrange(
                                "p (c e) -> p c e", e=64)
                            nc.gpsimd.dma_gather(
                                out_ap=g3, in_ap=tables[li][:],
                                idxs_ap=idx[:, 64 * jq:64 * jq + 64],
                                num_idxs=NIB, num_idxs_reg=NIB,
                                elem_size=64, queue_num=q)
                        # select 8-of-64 via one-hot masks from subid
                        sel = stage.tile([128, CH * FD], dt.float32, tag="sel")
                        msk = stage.tile([128, CH], dt.float32, tag="msk")
                        tmp = stage.tile([128, CH * FD], dt.float32, tag="tmp")
                        ck3 = ck[:].rearrange("p (c e) -> p c e", e=64)
                        sel3 = sel[:].rearrange("p (c f) -> p c f", f=FD)
                        tmp3 = tmp[:].rearrange("p (c f) -> p c f", f=FD)
                        for sidx in range(8):
                            nc.vector.tensor_scalar(
                                out=msk[:], in0=subid[:, c0:c0 + CH],
                                scalar1=float(sidx), scalar2=None,
                                op0=AO.is_equal)
                            mb = msk[:].rearrange(
                                "p (c o) -> p c o", o=1).to_broadcast(
                                [128, CH, FD])
                            blkv = ck3[:, :, sidx * FD:(sidx + 1) * FD]
                            if sidx == 0:
                                nc.vector.tensor_mul(out=sel3, in0=blkv, in1=mb)
                            else:
                                nc.vector.tensor_mul(out=tmp3, in0=blkv, in1=mb)
                                nc.vector.tensor_add(out=sel3, in0=sel3,
                                                     in1=tmp3)
                        # reduce chunk pieces into out
                        c1r = min(c0 + CH, creal)
                        if c1r <= c0:
                            continue
                        gA = (c0 - co) // s
                        gF0 = (c0 - co + s - 1) // s
                        gF1 = (c1r - co) // s
                        # head partial piece (segment gA continues from
                        # previous chunk or starts mid-chunk)
                        if gA < gF0:
                            a = 0
                            b = min(co + (gA + 1) * s, c1r) - c0
                            hv = sel[:, a * FD:b * FD].rearrange(
                                "p (o w) -> p o w", o=1)
                            tree_sum(hv, 1, b - a, b - a)
                            nc.vector.tensor_add(
                                out=out3[:, go + gA:go + gA + 1, :],
                                in0=out3[:, go + gA:go + gA + 1, :],
                                in1=hv[:, :, 0:FD])
                        # full segments
                        if gF1 > gF0:
                            a = co + gF0 * s - c0
                            nfull = gF1 - gF0
                            fv = sel[:, a * FD:(a + nfull * s) * FD].rearrange(
                                "p (n w) -> p n w", n=nfull)
                            tree_sum(fv, nfull, s, s)
                            nc.vector.tensor_add(
                                out=out3[:, go + gF0:go + gF1, :],
                                in0=out3[:, go + gF0:go + gF1, :],
                                in1=fv[:, :, 0:FD])
                        # tail partial piece
                        if gF1 >= gF0 and co + gF1 * s < c1r and gF1 < ng:
                            a = co + gF1 * s - c0
                            b = c1r - c0
                            tv = sel[:, a * FD:b * FD].rearrange(
                                "p (o w) -> p o w", o=1)
                            tree_sum(tv, 1, b - a, b - a)
                            nc.vector.tensor_add(
                                out=out3[:, go + gF1:go + gF1 + 1, :],
                                in0=out3[:, go + gF1:go + gF1 + 1, :],
                                in1=tv[:, :, 0:FD])
                    if tail is not None:
                        tail(go, ng)
                if tail is None:
                    # self-loop: own x*dinv contribution
                    nc.vector.tensor_add(out=out[:, :Gtot * FD],
                                         in0=out[:, :Gtot * FD], in1=ycur[:])

            # ---- chunked (per rank-group range) elementwise helpers ----
            TM = 32  # groups per sub-chunk (scratch-bounded)

            def cview(t, W, c0, nch):
                return t[:, c0 * W:(c0 + nch) * W].rearrange(
                    "p (g w) -> p g w", w=W)

            def cdinv(W, c0, nch):
                return dinv[:, c0:c0 + nch].rearrange(
                    "p (g o) -> p g o", o=1).to_broadcast([128, nch, W])

            def cadd_ycur(region, c0, nch):
                r3 = cview(region, FD, c0, nch)
                nc.vector.tensor_add(out=r3, in0=r3,
                                     in1=cview(ycur, FD, c0, nch))

            def cscale(region, W, c0, nch):
                r3 = cview(region, W, c0, nch)
                nc.vector.tensor_mul(out=r3, in0=r3, in1=cdinv(W, c0, nch))

            def cmatmul(src, Fin, Fout, wap, out, c0, nch):
                o3 = cview(out, Fout, c0, nch)
                t3 = tmat[:, :nch * Fout].rearrange("p (g w) -> p g w", w=Fout)
                s3 = cview(src, Fin, c0, nch)
                for fi in range(Fin):
                    sb = s3[:, :, fi:fi + 1].to_broadcast([128, nch, Fout])
                    wrow = wap[:, fi * Fout:(fi + 1) * Fout].rearrange(
                        "p (o w) -> p o w", o=1).to_broadcast([128, nch, Fout])
                    if fi == 0:
                        nc.vector.tensor_mul(out=o3, in0=sb, in1=wrow)
                    else:
                        nc.vector.tensor_mul(out=t3, in0=sb, in1=wrow)
                        nc.vector.tensor_add(out=o3, in0=o3, in1=t3)

            def cbias_relu(region, W, bap, c0, nch, relu=True):
                r3 = cview(region, W, c0, nch)
                bb = bap.rearrange("p (o w) -> p o w", o=1).to_broadcast(
                    [128, nch, W])
                nc.vector.tensor_add(out=r3, in0=r3, in1=bb)
                if relu:
                    nc.vector.tensor_relu(out=r3, in_=r3)

            def cwriteback(region, li, no, ng):
                shard_ap = shards[li][:].rearrange(
                    "(p g) f -> p g f", g=Gtot)[:, no:no + ng, :]
                nc.sync.dma_start(out=shard_ap,
                                  in_=cview(region, FD, no, ng))

            def tail_L0(no, ng):
                for c0 in range(no, no + ng, TM):
                    nch = min(TM, no + ng - c0)
                    cadd_ycur(agg, c0, nch)
                    cscale(agg, FD, c0, nch)
                    cmatmul(agg, FD, HD, cW0, xcur, c0, nch)
                    cbias_relu(xcur, HD, cb0, c0, nch)
                    cmatmul(xcur, HD, FD, cW1, agg, c0, nch)
                    cscale(agg, FD, c0, nch)
                    nc.vector.tensor_copy(out=cview(ycur, FD, c0, nch),
                                          in_=cview(agg, FD, c0, nch))
                cwriteback(agg, 1, no, ng)

            def tail_L1(no, ng):
                for c0 in range(no, no + ng, TM):
                    nch = min(TM, no + ng - c0)
                    cadd_ycur(agg, c0, nch)
                    cscale(agg, FD, c0, nch)
                    cbias_relu(agg, FD, cb1, c0, nch)
                    cmatmul(agg, FD, FD, wr[:, 552:616], xcur, c0, nch)
                    cscale(xcur, FD, c0, nch)
                    nc.vector.tensor_copy(out=cview(ycur, FD, c0, nch),
                                          in_=cview(xcur, FD, c0, nch))
                cwriteback(xcur, 2, no, ng)

            # y0 = dinv * x0 -> ycur, table
            x0r = agg[:, :Gtot * FD]
            scale_by_dinv(x0r, FD)
            nc.vector.tensor_copy(out=ycur[:], in_=x0r)
            writeback_allgather(x0r, 0)

            # ---------------- Layer 0 ----------------
            gather_layer(0, agg, tail=tail_L0)
            collective(1)

            # ---------------- Layer 1 ----------------
            gather_layer(1, agg, tail=tail_L1)
            collective(2)

            # ---------------- Layer 2 ----------------
            gather_layer(2, xcur)
            aggr3 = xcur[:, :Gtot * FD].rearrange("p (g w) -> p g w", w=FD)
            nc.vector.tensor_copy(out=fin[:].rearrange("p (g o) -> p g o", o=1),
                                  in_=aggr3[:, :, 0:1])
            nc.vector.tensor_mul(out=fin[:], in0=fin[:], in1=dinv[:])
            nc.scalar.activation(out=fin[:], in_=fin[:], func=AF.Tanh,
                                 bias=cb2)
            nc.sync.dma_start(out=out_d.rearrange("(p g) -> p g", g=Gtot),
                              in_=fin[:])
    nc.compile()
    return nc


def kernel(**inputs):
    from concourse.bass_utils import run_bass_kernel_spmd

    plan, idx16, sub, dinvT, featU, Wcomb, pirow = _preprocess(inputs)
    W0 = np.asarray(inputs["W0"], np.float32)
    b0 = np.asarray(inputs["b0"], np.float32)
    W1 = np.asarray(inputs["W1"], np.float32)
    b1 = np.asarray(inputs["b1"], np.float32)
    W2 = np.asarray(inputs["W2"], np.float32)
    b2 = np.asarray(inputs["b2"], np.float32)

    wrows = np.concatenate([W0.reshape(-1), b0, W1.reshape(-1), b1,
                            np.pad(W2[:, 0:1], ((0, 0), (0, 7))).reshape(-1),
                            np.pad(b2, (0, 7))])
    wrows_t = np.tile(wrows[None, :], (128, 1)).astype(np.float32)

    nc = _build_program(plan)

    in_maps = []
    for c in range(NCORE):
        in_maps.append({
            "featUT": np.ascontiguousarray(featU[c].T),
            "idx16": idx16[c],
            "subid": sub[c],
            "dinvt": dinvT[c],
            "wcomb": Wcomb,
            "wrows": wrows_t,
        })
    trace = False
    try:
        from antenv.axon_hooks import get_axon_ntff_profile_hook
        trace = get_axon_ntff_profile_hook() is not None
    except Exception:
        trace = False
    res = run_bass_kernel_spmd(nc, in_maps, list(range(NCORE)), trace=trace)
    global LAST_EXEC_NS, LAST_RES
    LAST_EXEC_NS = res.exec_time_ns
    LAST_RES = res
    allv = np.concatenate([res.results[c]["outv"] for c in range(NCORE)])
    return allv[pirow].astype(np.float32)
